# revision 19
# baseline (speedup 1.0000x reference)
"""DenseEnergyLoss Trainium2 kernel (ROI-gathered).

loss = WEIGHT * (-1/n) * sum_{k,i,j} A'[k,i] * G[i,j] * B'[k,j]

where (per image):
  f[i]  = [x/50, y/50, r/15, g/15, b/15]          (5-dim feature per pixel)
  G[i,j] = exp(f_i . f_j)                          (symmetric)
  e[i]  = exp(-0.5 |f_i|^2)
  B'[k,i] = seg_r[k,i] * e[i]
  A'[k,i] = seg_r[k,i] * gate[i] * e[i]
so that A' G B' == seg_r * gate * kern * seg_r with kern the bilateral kernel.

KEY REDUCTION: seg_r = segs * roi with roi in {0,1}, so pixels with roi==0
have B' == A' == 0 exactly and contribute nothing to the quadratic form
(gate==1 unlabeled pixels with roi==0 still have A' = B'*gate = 0). Only the
~2032 active pixels per image matter -> gather them and pad to P=2048.
This shrinks G from 4096^2 to 2048^2: exactly 4x less PE and ACT work.

Sharding: 2 cores per image (4 images x 8 cores). G is processed in
[128 x 512] tiles; symmetry halves the tile count: for column band b
(512 wide) only row blocks pb < 4*(b+1) are computed. Each G tile feeds one
accumulating matmul whose stationary packs [B'^T | A'^T] (42 cols): the B'
half covers the lower-left triangle term (dotted against A' at the end), the
A' half covers the transposed upper-right term (dotted against B'), valid
only for blocks strictly above the diagonal super-tile (s < 2b, uniform
across cores thanks to the parity split: core half h owns global blocks
2s+h).

Device pipeline per tile pair: PE matmul (c=15 bf16 hi/lo-compensated feature
contraction, row-group packed x2) -> ScalarE exp ([128,1024] PSUM->SBUF bf16)
-> PE matmul x2 (col-strip packed at cols 0/64, bf16) accumulating into a
per-band PSUM bank -> DVE fused multiply-reduce per band (against a bf16
[A';B'] replica). Host sums the per-core [128, NB] partials.

DMAs are issued smallest-needed-first across the Sync/Vector/Scalar/GpSimd
queues so the first matmul can start right after the framework preamble.
"""

import numpy as np
import ml_dtypes

WEIGHT = 1e-07
SIGMA_RGB = 15.0
SIGMA_XY_EFF = 50.0  # SIGMA_XY * SCALE
IGNORE_LABEL = 255

N_IMG = 4
K_CLS = 21
H_DS = 64
P_FULL = H_DS * H_DS  # 4096 downsampled pixels
BAND = 512
BLK = 128
W2 = 2 * K_CLS  # 42: combined [B'|A'] stationary width

BF16 = ml_dtypes.bfloat16

_CACHE = {}


def _rg(s):
    # row-group for mm1 packing: pairs alternate {0,1} / {2,3}
    return 2 * ((s // 2) % 2) + (s % 2)


def _build_program(n_lslot):
    """n_lslot: local row-block slots per core. P_act = 256 * n_lslot,
    NB = n_lslot // 2 bands of 512. n_lslot must be even."""
    import concourse.bacc as bacc
    import concourse.tile as tile
    from concourse import mybir

    f32 = mybir.dt.float32
    bf16 = mybir.dt.bfloat16

    nb = n_lslot // 2
    p_act = 256 * n_lslot

    nc = bacc.Bacc("TRN2", target_bir_lowering=False, debug=False)

    # mov/stat sources hold the 15 feature rows replicated twice (30 rows)
    # so one grouped-partition DMA can fill two SBUF row groups at once.
    mov_d = nc.dram_tensor("mov_src", [30, p_act], bf16, kind="ExternalInput")
    stat_d = nc.dram_tensor("stat_src", [30, n_lslot * BLK], bf16, kind="ExternalInput")
    bapt_d = nc.dram_tensor("bapt", [128, n_lslot * W2], bf16, kind="ExternalInput")
    abrep_d = nc.dram_tensor("abrep_src", [128, p_act], bf16, kind="ExternalInput")
    acc_d = nc.dram_tensor("acc_out", [128, nb], f32, kind="ExternalOutput")

    with tile.TileContext(nc) as tc:
        with (
            tc.tile_pool(name="const", bufs=1) as cpool,
            tc.tile_pool(name="gpsum", bufs=3, space="PSUM") as gpool,
            tc.tile_pool(name="accpsum", bufs=2, space="PSUM") as apool,
            tc.tile_pool(name="gsb", bufs=4) as gsbpool,
            tc.tile_pool(name="scr", bufs=2) as scrpool,
            tc.tile_pool(name="tsb", bufs=2) as tpool,
        ):
            ft_stat = cpool.tile([128, n_lslot * BLK], bf16, tag="ftstat")
            ft_mov = cpool.tile([128, p_act], bf16, tag="ftmov")
            bapt = cpool.tile([128, n_lslot * W2], bf16, tag="bapt")
            abrep = cpool.tile([128, p_act], bf16, tag="abrep")
            acc = cpool.tile([128, nb], f32, tag="acc")
            ebase = cpool.tile([128, 1024], f32, tag="ebase")

            # e-base for the GPSIMD pow-based exp offload (e^t == pow(e, t))
            nc.gpsimd.memset(ebase[:], float(np.e))

            # One memset per PSUM accumulator buffer (never re-zeroed:
            # per-band resets come from start=True on the first mm2 pair,
            # and rows no mm2 ever writes stay zero from here on).
            m_ba_pre = []
            for _ in range(min(2, nb)):
                m = apool.tile([128, BAND], f32, tag="mba")
                nc.vector.memset(m[:], 0.0)
                m_ba_pre.append(m)
            # Bands alternate between these two tile objects (not fresh
            # pool allocations: initialization must persist across bands).

            # --- input DMAs, ordered by first need ------------------------
            # Bands run largest (b = nb-1) first; band b's pair p uses slots
            # 2p, 2p+1 (row groups given by _rg) and mov columns of band b.
            # Plain per-row-group partition slices (baseline-proven APs);
            # the 30-row sources allow rg pairs to read distinct DRAM rows.
            def mov_rg(rg, c0, c1):
                src_r = 15 * (rg % 2)
                return (
                    ft_mov[32 * rg : 32 * rg + 15, c0:c1],
                    mov_d[src_r : src_r + 15, c0:c1],
                )

            def stat_rg(rg):
                src_r = 15 * (rg % 2)
                return (
                    ft_stat[32 * rg : 32 * rg + 15, :],
                    stat_d[src_r : src_r + 15, :],
                )

            def abrep_q(c0, c1):
                return abrep[:, c0:c1], abrep_d[:, c0:c1]

            bands_desc = list(reversed(range(nb)))  # e.g. [3,2,1,0]
            b_last = bands_desc[0]
            lb0, lb1 = b_last * BAND, (b_last + 1) * BAND
            rest_hi = lb0  # mov columns 0:rest_hi still to load after band b_last

            # Scalar: ONLY the most critical load (stat row group 0, first
            # two slots first), then it must go idle for the ACT table load
            # + exps.
            nc.scalar.dma_start(
                ft_stat[0:15, 0 : 2 * BLK], stat_d[0:15, 0 : 2 * BLK]
            )
            if n_lslot > 2:
                nc.scalar.dma_start(
                    ft_stat[0:15, 2 * BLK :], stat_d[0:15, 2 * BLK :]
                )

            # Sync: first band's mov rg0/rg1, bapt in slot-pair chunks, the
            # remaining rg0/rg1 mov, then abrep.
            nc.sync.dma_start(*mov_rg(0, lb0, lb1))
            nc.sync.dma_start(*mov_rg(1, lb0, lb1))
            nc.sync.dma_start(bapt[:, 0 : 2 * W2], bapt_d[:, 0 : 2 * W2])
            if n_lslot > 2:
                nc.sync.dma_start(
                    bapt[:, 2 * W2 : 4 * W2], bapt_d[:, 2 * W2 : 4 * W2]
                )
            if rest_hi > 0:
                nc.sync.dma_start(*mov_rg(0, 0, rest_hi))
                nc.sync.dma_start(*mov_rg(1, 0, rest_hi))
            if n_lslot > 4:
                nc.sync.dma_start(bapt[:, 4 * W2 :], bapt_d[:, 4 * W2 :])
            if nb > 2:
                c0 = bands_desc[2] * BAND
                nc.sync.dma_start(*abrep_q(c0, c0 + BAND))
            if nb > 3:
                for bx in bands_desc[3:]:
                    nc.sync.dma_start(*abrep_q(bx * BAND, (bx + 1) * BAND))

            # GpSimd: (after the ebase memset) the other stat row groups
            # (rg1 slots 0,1 first: they gate the first matmul), first
            # band's mov rg2/rg3, first-needed abrep, rest.
            nc.gpsimd.dma_start(
                ft_stat[32:47, 0 : 2 * BLK], stat_d[15:30, 0 : 2 * BLK]
            )
            if n_lslot > 2:
                nc.gpsimd.dma_start(
                    ft_stat[32:47, 2 * BLK :], stat_d[15:30, 2 * BLK :]
                )
            nc.gpsimd.dma_start(*stat_rg(2))
            nc.gpsimd.dma_start(*stat_rg(3))
            nc.gpsimd.dma_start(*mov_rg(2, lb0, lb1))
            nc.gpsimd.dma_start(*mov_rg(3, lb0, lb1))
            nc.gpsimd.dma_start(*abrep_q(lb0, lb1))
            if rest_hi > 0:
                nc.gpsimd.dma_start(*mov_rg(2, 0, rest_hi))
                nc.gpsimd.dma_start(*mov_rg(3, 0, rest_hi))
            if nb > 1:
                c0 = bands_desc[1] * BAND
                nc.gpsimd.dma_start(*abrep_q(c0, c0 + BAND))

            # --- main loop: largest band first ----------------------------
            # Per band: no PSUM memset -- the first-emitted mm2 pair uses
            # start=True to zero its strips, and the band-0 A-side garbage
            # rows are killed by host-zeroed abrep columns. The diagonal
            # pair's exp (bands >= 1) is offloaded to GpSimd as pow(e, t)
            # after a DVE PSUM->SBUF copy; it is processed FIRST (long
            # latency chain) and its mm2 is emitted LAST.
            for bi, b in enumerate(reversed(range(nb))):
                m_ba = m_ba_pre[bi % len(m_ba_pre)]
                n_pairs = b + 1
                offload = b >= 1
                seq = ([b] + list(range(b))) if offload else list(range(n_pairs))

                gp_t = {}
                gsb_t = {}

                def emit_mm1(p, b=b, gp_t=gp_t):
                    gp = gpool.tile([128, 1024], f32, tag="g")
                    gp_t[p] = gp
                    for t in range(2):
                        s = 2 * p + t
                        rg = _rg(s)
                        nc.tensor.matmul(
                            gp[:, t * BAND : (t + 1) * BAND],
                            ft_stat[32 * rg : 32 * rg + 15, s * BLK : (s + 1) * BLK],
                            ft_mov[32 * rg : 32 * rg + 15, b * BAND : (b + 1) * BAND],
                            start=True,
                            stop=True,
                            tile_position=(32 * rg, 0),
                        )

                def emit_exp(p, use_gpsimd, gp_t=gp_t, gsb_t=gsb_t):
                    gsb = gsbpool.tile([128, 1024], bf16, tag="gsb")
                    gsb_t[p] = gsb
                    gp = gp_t.pop(p)
                    if use_gpsimd:
                        tsb = tpool.tile([128, 1024], f32, tag="tsb")
                        nc.vector.tensor_scalar_add(tsb[:], gp[:], 0.0)
                        nc.gpsimd.tensor_tensor(
                            gsb[:], ebase[:], tsb[:], mybir.AluOpType.pow
                        )
                    else:
                        nc.scalar.activation(
                            gsb[:], gp[:], mybir.ActivationFunctionType.Exp
                        )

                # mm1 + exp with one-pair emission lookahead so consecutive
                # pairs' mm1s (disjoint PE row groups) overlap in the array
                emit_mm1(seq[0])
                for i, p in enumerate(seq):
                    if i + 1 < len(seq):
                        emit_mm1(seq[i + 1])
                    emit_exp(p, use_gpsimd=(offload and p == b))

                # mm2: sub-diagonal pairs first, offloaded diagonal last
                mm2_seq = (list(range(b)) + [b]) if offload else list(range(n_pairs))
                for j, p in enumerate(mm2_seq):
                    gsb = gsb_t.pop(p)
                    for t in range(2):
                        s = 2 * p + t
                        w = W2 if s < 2 * b else K_CLS  # A-side only above diag
                        col = 64 * (s % 2)
                        nc.tensor.matmul(
                            m_ba[col : col + w, :],
                            bapt[:, s * W2 : s * W2 + w],
                            gsb[:, t * BAND : (t + 1) * BAND],
                            start=(j == 0),
                            stop=(j == len(mm2_seq) - 1 and t == 1),
                            tile_position=(0, col),
                            skip_group_check=True,
                        )

                sc0 = scrpool.tile([128, BAND], f32, tag="sc")
                nc.vector.tensor_tensor(
                    sc0[:],
                    m_ba[:],
                    abrep[:, b * BAND : (b + 1) * BAND],
                    mybir.AluOpType.mult,
                )
                nc.vector.reduce_sum(
                    acc[:, b : b + 1], sc0[:], axis=mybir.AxisListType.X
                )
                nc.sync.dma_start(acc_d[:, b : b + 1], acc[:, b : b + 1])

    nc.compile()
    return nc


def _host_prep(images, segmentations, ROIs, seg_label):
    """Returns (n_lslot, per-core input dicts). Core c -> image c//2, half
    c%2. Core half h owns global row blocks 2s+h of the gathered pixel set,
    s in [0, n_lslot)."""
    imgs = images[:, :, ::2, ::2].astype(np.float64)  # [N,3,64,64]
    segs = (
        segmentations.astype(np.float64)
        .reshape(N_IMG, K_CLS, H_DS, 2, H_DS, 2)
        .mean(axis=(3, 5))
    )  # [N,21,64,64]
    rois = ROIs[:, ::2, ::2].astype(np.float64)  # [N,64,64]
    lbl = seg_label[:, 0, ::2, ::2]  # [N,64,64] int32
    unlabel = lbl == IGNORE_LABEL

    seg_max = segs.max(axis=1)
    gate = rois - seg_max
    gate = np.where(unlabel, 1.0, gate)
    gate = np.maximum(gate, 0.0)  # [N,64,64]
    seg_r = segs * rois[:, None]  # [N,21,64,64]

    yy, xx = np.meshgrid(
        np.arange(H_DS, dtype=np.float64),
        np.arange(H_DS, dtype=np.float64),
        indexing="ij",
    )
    f = np.concatenate(
        [
            np.broadcast_to((xx / SIGMA_XY_EFF).reshape(1, 1, P_FULL), (N_IMG, 1, P_FULL)),
            np.broadcast_to((yy / SIGMA_XY_EFF).reshape(1, 1, P_FULL), (N_IMG, 1, P_FULL)),
            imgs.reshape(N_IMG, 3, P_FULL) / SIGMA_RGB,
        ],
        axis=1,
    )  # [N, 5, P_FULL]
    sq = (f * f).sum(axis=1)  # [N, P_FULL]
    e = np.exp(-0.5 * sq)  # [N, P_FULL]

    Bp_full = seg_r.reshape(N_IMG, K_CLS, P_FULL) * e[:, None, :]  # [N,21,P]
    Ap_full = Bp_full * gate.reshape(N_IMG, 1, P_FULL)

    # Gather ROI-active pixels (B' == A' == 0 elsewhere: exact reduction).
    act_idx = [np.flatnonzero(rois[i].ravel()) for i in range(N_IMG)]
    n_max = max(len(ix) for ix in act_idx)
    n_lslot = max(2, 2 * ((n_max + 511) // 512))  # even, P_act >= n_max
    p_act = 256 * n_lslot

    f32 = np.float32
    in_maps = []
    for core in range(8):
        img_i = core // 2
        half = core % 2
        ix = act_idx[img_i]
        n = len(ix)

        fi = np.zeros((5, p_act), np.float64)
        fi[:, :n] = f[img_i][:, ix]
        Bp = np.zeros((K_CLS, p_act), np.float64)
        Bp[:, :n] = Bp_full[img_i][:, ix]
        Ap = np.zeros((K_CLS, p_act), np.float64)
        Ap[:, :n] = Ap_full[img_i][:, ix]

        f_32 = fi.astype(f32)
        f_hi = f_32.astype(BF16)
        f_lo = (f_32 - f_hi.astype(f32)).astype(BF16)  # [5,p_act] each

        # mov_src: [hi; hi; lo] rows, replicated x2 so one grouped DMA can
        # fill two SBUF row groups (the DMA header reads it twice for 4).
        mov_15 = np.concatenate([f_hi, f_hi, f_lo], axis=0)  # [15, p_act]
        mov_src = np.concatenate([mov_15, mov_15], axis=0)  # [30, p_act]

        # stat_src: local slot s holds [hi; lo; hi] of global block 2s+half.
        # bapt: [B'^T | A'^T] of the same block.
        stat_src = np.zeros((15, n_lslot * BLK), BF16)
        bapt = np.zeros((128, n_lslot * W2), BF16)
        BpT = np.ascontiguousarray(Bp.T).astype(BF16)  # [p_act, 21]
        ApT = np.ascontiguousarray(Ap.T).astype(BF16)  # [p_act, 21]
        for s in range(n_lslot):
            blk = 2 * s + half
            cols = slice(s * BLK, (s + 1) * BLK)
            pix = slice(blk * BLK, (blk + 1) * BLK)
            stat_src[0:5, cols] = f_hi[:, pix]
            stat_src[5:10, cols] = f_lo[:, pix]
            stat_src[10:15, cols] = f_hi[:, pix]
            bapt[:, s * W2 : s * W2 + K_CLS] = BpT[pix]
            bapt[:, s * W2 + K_CLS : (s + 1) * W2] = ApT[pix]

        # abrep_src: rows 0-20 A', 21-41 B', 42-63 zero, rows 64-127 copy.
        # Band-0 columns of the B' rows are zeroed: the device never writes
        # band 0's A-side PSUM rows (no sub-diagonal tiles there), so this
        # kills the garbage they would otherwise multiply.
        abrep_src = np.zeros((128, p_act), BF16)
        abrep_src[0:K_CLS] = Ap.astype(BF16)
        abrep_src[K_CLS:W2] = Bp.astype(BF16)
        abrep_src[K_CLS:W2, 0:BAND] = 0
        abrep_src[64:] = abrep_src[:64]

        in_maps.append(
            {
                "mov_src": mov_src,
                "stat_src": np.concatenate([stat_src, stat_src], axis=0),
                "bapt": bapt,
                "abrep_src": abrep_src,
            }
        )
    return n_lslot, in_maps


def _get_program(n_lslot):
    key = ("nc", n_lslot)
    if key not in _CACHE:
        _CACHE[key] = _build_program(n_lslot)
    return _CACHE[key]


def _install_profile_hook():
    """Best-effort registration of the axon NTFF profile hook so that
    trace=True works (used by test harness, not the plain kernel path)."""
    import sys
    import types

    if "antenv.axon_hooks" in sys.modules:
        return
    try:
        from trn_agent_boot.trn_boot import _ntff_profile_via_ctypes

        hook = _ntff_profile_via_ctypes("/opt/axon/libaxon_pjrt.so")
        mod = types.ModuleType("antenv.axon_hooks")
        mod.get_axon_ntff_profile_hook = lambda: hook
        sys.modules["antenv.axon_hooks"] = mod
    except Exception:
        pass


def kernel(images, segmentations, ROIs, seg_label, _trace=False, _tmpdir=None):
    from concourse import bass_utils

    n_lslot, in_maps = _host_prep(images, segmentations, ROIs, seg_label)
    nc = _get_program(n_lslot)
    if _trace:
        _install_profile_hook()
        bass_utils.upload_artifacts = lambda tmpdir: f"local:{tmpdir}"
    res = bass_utils.run_bass_kernel_spmd(
        nc, in_maps, list(range(8)), trace=_trace, tmpdir=_tmpdir
    )
    total = 0.0
    for r in res.results:
        total += r["acc_out"].astype(np.float64).sum()
    loss = np.float32(-WEIGHT / N_IMG * total)
    if _trace:
        return np.array([loss], np.float32), res
    return np.array([loss], np.float32)


# revision 20
# speedup vs baseline: 17.3244x; 17.3244x over previous
"""DenseEnergyLoss Trainium2 kernel (ROI-gathered).

loss = WEIGHT * (-1/n) * sum_{k,i,j} A'[k,i] * G[i,j] * B'[k,j]

where (per image):
  f[i]  = [x/50, y/50, r/15, g/15, b/15]          (5-dim feature per pixel)
  G[i,j] = exp(f_i . f_j)                          (symmetric)
  e[i]  = exp(-0.5 |f_i|^2)
  B'[k,i] = seg_r[k,i] * e[i]
  A'[k,i] = seg_r[k,i] * gate[i] * e[i]
so that A' G B' == seg_r * gate * kern * seg_r with kern the bilateral kernel.

KEY REDUCTION: seg_r = segs * roi with roi in {0,1}, so pixels with roi==0
have B' == A' == 0 exactly and contribute nothing to the quadratic form
(gate==1 unlabeled pixels with roi==0 still have A' = B'*gate = 0). Only the
~2032 active pixels per image matter -> gather them and pad to P=2048.
This shrinks G from 4096^2 to 2048^2: exactly 4x less PE and ACT work.

Sharding: 2 cores per image (4 images x 8 cores). G is processed in
[128 x 512] tiles; symmetry halves the tile count: for column band b
(512 wide) only row blocks pb < 4*(b+1) are computed. Each G tile feeds one
accumulating matmul whose stationary packs [B'^T | A'^T] (42 cols): the B'
half covers the lower-left triangle term (dotted against A' at the end), the
A' half covers the transposed upper-right term (dotted against B'), valid
only for blocks strictly above the diagonal super-tile (s < 2b, uniform
across cores thanks to the parity split: core half h owns global blocks
2s+h).

Device pipeline per tile pair: PE matmul (c=15 bf16 hi/lo-compensated feature
contraction, row-group packed x2) -> ScalarE exp ([128,1024] PSUM->SBUF bf16)
-> PE matmul x2 (col-strip packed at cols 0/64, bf16) accumulating into a
per-band PSUM bank -> DVE fused multiply-reduce per band (against a bf16
[A';B'] replica). Host sums the per-core [128, NB] partials.

DMAs are issued smallest-needed-first across the Sync/Vector/Scalar/GpSimd
queues so the first matmul can start right after the framework preamble.
"""

import numpy as np
import ml_dtypes

WEIGHT = 1e-07
SIGMA_RGB = 15.0
SIGMA_XY_EFF = 50.0  # SIGMA_XY * SCALE
IGNORE_LABEL = 255

N_IMG = 4
K_CLS = 21
H_DS = 64
P_FULL = H_DS * H_DS  # 4096 downsampled pixels
BAND = 512
BLK = 128
W2 = 2 * K_CLS  # 42: combined [B'|A'] stationary width

BF16 = ml_dtypes.bfloat16

_CACHE = {}


def _rg(s):
    # row-group for mm1 packing: pairs alternate {0,1} / {2,3}
    return 2 * ((s // 2) % 2) + (s % 2)


def _build_program(n_lslot):
    """n_lslot: local row-block slots per core. P_act = 256 * n_lslot,
    NB = n_lslot // 2 bands of 512. n_lslot must be even."""
    import concourse.bacc as bacc
    import concourse.tile as tile
    from concourse import mybir

    f32 = mybir.dt.float32
    bf16 = mybir.dt.bfloat16

    nb = n_lslot // 2
    p_act = 256 * n_lslot

    nc = bacc.Bacc("TRN2", target_bir_lowering=False, debug=False)

    # mov/stat sources hold the 15 feature rows replicated twice (30 rows)
    # so one grouped-partition DMA can fill two SBUF row groups at once.
    mov_d = nc.dram_tensor("mov_src", [30, p_act], bf16, kind="ExternalInput")
    stat_d = nc.dram_tensor("stat_src", [30, n_lslot * BLK], bf16, kind="ExternalInput")
    bapt_d = nc.dram_tensor("bapt", [128, n_lslot * W2], bf16, kind="ExternalInput")
    abrep_d = nc.dram_tensor("abrep_src", [128, p_act], bf16, kind="ExternalInput")
    acc_d = nc.dram_tensor("acc_out", [128, nb], f32, kind="ExternalOutput")

    with tile.TileContext(nc) as tc:
        with (
            tc.tile_pool(name="const", bufs=1) as cpool,
            tc.tile_pool(name="gpsum", bufs=3, space="PSUM") as gpool,
            tc.tile_pool(name="accpsum", bufs=2, space="PSUM") as apool,
            tc.tile_pool(name="gsb", bufs=4) as gsbpool,
            tc.tile_pool(name="scr", bufs=2) as scrpool,
            tc.tile_pool(name="tsb", bufs=2) as tpool,
        ):
            ft_stat = cpool.tile([128, n_lslot * BLK], bf16, tag="ftstat")
            ft_mov = cpool.tile([128, p_act], bf16, tag="ftmov")
            bapt = cpool.tile([128, n_lslot * W2], bf16, tag="bapt")
            abrep = cpool.tile([128, p_act], bf16, tag="abrep")
            acc = cpool.tile([128, nb], f32, tag="acc")
            ebase = cpool.tile([128, 1024], f32, tag="ebase")

            # e-base for the GPSIMD pow-based exp offload (e^t == pow(e, t))
            nc.gpsimd.memset(ebase[:], float(np.e))

            # One memset per PSUM accumulator buffer (never re-zeroed:
            # per-band resets come from start=True on the first mm2 pair,
            # and rows no mm2 ever writes stay zero from here on).
            m_ba_pre = []
            for _ in range(min(2, nb)):
                m = apool.tile([128, BAND], f32, tag="mba")
                nc.vector.memset(m[:], 0.0)
                m_ba_pre.append(m)
            # Bands alternate between these two tile objects (not fresh
            # pool allocations: initialization must persist across bands).

            # --- input DMAs, ordered by first need ------------------------
            # Bands run largest (b = nb-1) first; band b's pair p uses slots
            # 2p, 2p+1 (row groups given by _rg) and mov columns of band b.
            # Plain per-row-group partition slices (baseline-proven APs);
            # the 30-row sources allow rg pairs to read distinct DRAM rows.
            def mov_rg(rg, c0, c1):
                src_r = 15 * (rg % 2)
                return (
                    ft_mov[32 * rg : 32 * rg + 15, c0:c1],
                    mov_d[src_r : src_r + 15, c0:c1],
                )

            def stat_rg(rg):
                src_r = 15 * (rg % 2)
                return (
                    ft_stat[32 * rg : 32 * rg + 15, :],
                    stat_d[src_r : src_r + 15, :],
                )

            def abrep_q(c0, c1):
                return abrep[:, c0:c1], abrep_d[:, c0:c1]

            bands_desc = list(reversed(range(nb)))  # e.g. [3,2,1,0]
            b_last = bands_desc[0]
            lb0, lb1 = b_last * BAND, (b_last + 1) * BAND
            rest_hi = lb0  # mov columns 0:rest_hi still to load after band b_last

            # Scalar: ONLY the most critical load (stat row group 0, first
            # two slots first), then it must go idle for the ACT table load
            # + exps.
            nc.scalar.dma_start(
                ft_stat[0:15, 0 : 2 * BLK], stat_d[0:15, 0 : 2 * BLK]
            )
            if n_lslot > 2:
                nc.scalar.dma_start(
                    ft_stat[0:15, 2 * BLK :], stat_d[0:15, 2 * BLK :]
                )

            # Sync: first band's mov rg0/rg1, bapt in slot-pair chunks, the
            # remaining rg0/rg1 mov, then abrep.
            nc.sync.dma_start(*mov_rg(0, lb0, lb1))
            nc.sync.dma_start(*mov_rg(1, lb0, lb1))
            nc.sync.dma_start(bapt[:, 0 : 2 * W2], bapt_d[:, 0 : 2 * W2])
            if n_lslot > 2:
                nc.sync.dma_start(
                    bapt[:, 2 * W2 : 4 * W2], bapt_d[:, 2 * W2 : 4 * W2]
                )
            if rest_hi > 0:
                nc.sync.dma_start(*mov_rg(0, 0, rest_hi))
                nc.sync.dma_start(*mov_rg(1, 0, rest_hi))
            if n_lslot > 4:
                nc.sync.dma_start(bapt[:, 4 * W2 :], bapt_d[:, 4 * W2 :])
            if nb > 2:
                c0 = bands_desc[2] * BAND
                nc.sync.dma_start(*abrep_q(c0, c0 + BAND))
            if nb > 3:
                for bx in bands_desc[3:]:
                    nc.sync.dma_start(*abrep_q(bx * BAND, (bx + 1) * BAND))

            # GpSimd: (after the ebase memset) the other stat row groups
            # (rg1 slots 0,1 first: they gate the first matmul), first
            # band's mov rg2/rg3, first-needed abrep, rest.
            nc.gpsimd.dma_start(
                ft_stat[32:47, 0 : 2 * BLK], stat_d[15:30, 0 : 2 * BLK]
            )
            if n_lslot > 2:
                nc.gpsimd.dma_start(
                    ft_stat[32:47, 2 * BLK :], stat_d[15:30, 2 * BLK :]
                )
            nc.gpsimd.dma_start(*stat_rg(2))
            nc.gpsimd.dma_start(*stat_rg(3))
            nc.gpsimd.dma_start(*mov_rg(2, lb0, lb1))
            nc.gpsimd.dma_start(*mov_rg(3, lb0, lb1))
            nc.gpsimd.dma_start(*abrep_q(lb0, lb1))
            if rest_hi > 0:
                nc.gpsimd.dma_start(*mov_rg(2, 0, rest_hi))
                nc.gpsimd.dma_start(*mov_rg(3, 0, rest_hi))
            if nb > 1:
                c0 = bands_desc[1] * BAND
                nc.gpsimd.dma_start(*abrep_q(c0, c0 + BAND))

            # --- main loop: largest band first ----------------------------
            # Per band: no PSUM memset -- the first-emitted mm2 pair uses
            # start=True to zero its strips, and the band-0 A-side garbage
            # rows are killed by host-zeroed abrep columns. The diagonal
            # pair's exp (bands >= 1) is offloaded to GpSimd as pow(e, t)
            # after a DVE PSUM->SBUF copy; it is processed FIRST (long
            # latency chain) and its mm2 is emitted LAST.
            for bi, b in enumerate(reversed(range(nb))):
                m_ba = m_ba_pre[bi % len(m_ba_pre)]
                n_pairs = b + 1
                offload = False  # GpSimd pow measured ~170us/tile: unusable
                seq = ([b] + list(range(b))) if offload else list(range(n_pairs))

                gp_t = {}
                gsb_t = {}

                def emit_mm1(p, b=b, gp_t=gp_t):
                    gp = gpool.tile([128, 1024], f32, tag="g")
                    gp_t[p] = gp
                    for t in range(2):
                        s = 2 * p + t
                        rg = _rg(s)
                        nc.tensor.matmul(
                            gp[:, t * BAND : (t + 1) * BAND],
                            ft_stat[32 * rg : 32 * rg + 15, s * BLK : (s + 1) * BLK],
                            ft_mov[32 * rg : 32 * rg + 15, b * BAND : (b + 1) * BAND],
                            start=True,
                            stop=True,
                            tile_position=(32 * rg, 0),
                        )

                def emit_exp(p, use_gpsimd, gp_t=gp_t, gsb_t=gsb_t):
                    gsb = gsbpool.tile([128, 1024], bf16, tag="gsb")
                    gsb_t[p] = gsb
                    gp = gp_t.pop(p)
                    if use_gpsimd:
                        tsb = tpool.tile([128, 1024], f32, tag="tsb")
                        nc.vector.tensor_scalar_add(tsb[:], gp[:], 0.0)
                        nc.gpsimd.tensor_tensor(
                            gsb[:], ebase[:], tsb[:], mybir.AluOpType.pow
                        )
                    else:
                        nc.scalar.activation(
                            gsb[:], gp[:], mybir.ActivationFunctionType.Exp
                        )

                # mm1 + exp with one-pair emission lookahead so consecutive
                # pairs' mm1s (disjoint PE row groups) overlap in the array
                emit_mm1(seq[0])
                for i, p in enumerate(seq):
                    if i + 1 < len(seq):
                        emit_mm1(seq[i + 1])
                    emit_exp(p, use_gpsimd=(offload and p == b))

                # mm2: sub-diagonal pairs first, offloaded diagonal last
                mm2_seq = (list(range(b)) + [b]) if offload else list(range(n_pairs))
                for j, p in enumerate(mm2_seq):
                    gsb = gsb_t.pop(p)
                    for t in range(2):
                        s = 2 * p + t
                        w = W2 if s < 2 * b else K_CLS  # A-side only above diag
                        col = 64 * (s % 2)
                        nc.tensor.matmul(
                            m_ba[col : col + w, :],
                            bapt[:, s * W2 : s * W2 + w],
                            gsb[:, t * BAND : (t + 1) * BAND],
                            start=(j == 0),
                            stop=(j == len(mm2_seq) - 1 and t == 1),
                            tile_position=(0, col),
                            skip_group_check=True,
                        )

                sc0 = scrpool.tile([128, BAND], f32, tag="sc")
                nc.vector.tensor_tensor(
                    sc0[:],
                    m_ba[:],
                    abrep[:, b * BAND : (b + 1) * BAND],
                    mybir.AluOpType.mult,
                )
                nc.vector.reduce_sum(
                    acc[:, b : b + 1], sc0[:], axis=mybir.AxisListType.X
                )
                nc.sync.dma_start(acc_d[:, b : b + 1], acc[:, b : b + 1])

    nc.compile()
    return nc


def _host_prep(images, segmentations, ROIs, seg_label):
    """Returns (n_lslot, per-core input dicts). Core c -> image c//2, half
    c%2. Core half h owns global row blocks 2s+h of the gathered pixel set,
    s in [0, n_lslot)."""
    imgs = images[:, :, ::2, ::2].astype(np.float64)  # [N,3,64,64]
    segs = (
        segmentations.astype(np.float64)
        .reshape(N_IMG, K_CLS, H_DS, 2, H_DS, 2)
        .mean(axis=(3, 5))
    )  # [N,21,64,64]
    rois = ROIs[:, ::2, ::2].astype(np.float64)  # [N,64,64]
    lbl = seg_label[:, 0, ::2, ::2]  # [N,64,64] int32
    unlabel = lbl == IGNORE_LABEL

    seg_max = segs.max(axis=1)
    gate = rois - seg_max
    gate = np.where(unlabel, 1.0, gate)
    gate = np.maximum(gate, 0.0)  # [N,64,64]
    seg_r = segs * rois[:, None]  # [N,21,64,64]

    yy, xx = np.meshgrid(
        np.arange(H_DS, dtype=np.float64),
        np.arange(H_DS, dtype=np.float64),
        indexing="ij",
    )
    f = np.concatenate(
        [
            np.broadcast_to((xx / SIGMA_XY_EFF).reshape(1, 1, P_FULL), (N_IMG, 1, P_FULL)),
            np.broadcast_to((yy / SIGMA_XY_EFF).reshape(1, 1, P_FULL), (N_IMG, 1, P_FULL)),
            imgs.reshape(N_IMG, 3, P_FULL) / SIGMA_RGB,
        ],
        axis=1,
    )  # [N, 5, P_FULL]
    sq = (f * f).sum(axis=1)  # [N, P_FULL]
    e = np.exp(-0.5 * sq)  # [N, P_FULL]

    Bp_full = seg_r.reshape(N_IMG, K_CLS, P_FULL) * e[:, None, :]  # [N,21,P]
    Ap_full = Bp_full * gate.reshape(N_IMG, 1, P_FULL)

    # Gather ROI-active pixels (B' == A' == 0 elsewhere: exact reduction).
    act_idx = [np.flatnonzero(rois[i].ravel()) for i in range(N_IMG)]
    n_max = max(len(ix) for ix in act_idx)
    n_lslot = max(2, 2 * ((n_max + 511) // 512))  # even, P_act >= n_max
    p_act = 256 * n_lslot

    f32 = np.float32
    in_maps = []
    for core in range(8):
        img_i = core // 2
        half = core % 2
        ix = act_idx[img_i]
        n = len(ix)

        fi = np.zeros((5, p_act), np.float64)
        fi[:, :n] = f[img_i][:, ix]
        Bp = np.zeros((K_CLS, p_act), np.float64)
        Bp[:, :n] = Bp_full[img_i][:, ix]
        Ap = np.zeros((K_CLS, p_act), np.float64)
        Ap[:, :n] = Ap_full[img_i][:, ix]

        f_32 = fi.astype(f32)
        f_hi = f_32.astype(BF16)
        f_lo = (f_32 - f_hi.astype(f32)).astype(BF16)  # [5,p_act] each

        # mov_src: [hi; hi; lo] rows, replicated x2 so one grouped DMA can
        # fill two SBUF row groups (the DMA header reads it twice for 4).
        mov_15 = np.concatenate([f_hi, f_hi, f_lo], axis=0)  # [15, p_act]
        mov_src = np.concatenate([mov_15, mov_15], axis=0)  # [30, p_act]

        # stat_src: local slot s holds [hi; lo; hi] of global block 2s+half.
        # bapt: [B'^T | A'^T] of the same block.
        stat_src = np.zeros((15, n_lslot * BLK), BF16)
        bapt = np.zeros((128, n_lslot * W2), BF16)
        BpT = np.ascontiguousarray(Bp.T).astype(BF16)  # [p_act, 21]
        ApT = np.ascontiguousarray(Ap.T).astype(BF16)  # [p_act, 21]
        for s in range(n_lslot):
            blk = 2 * s + half
            cols = slice(s * BLK, (s + 1) * BLK)
            pix = slice(blk * BLK, (blk + 1) * BLK)
            stat_src[0:5, cols] = f_hi[:, pix]
            stat_src[5:10, cols] = f_lo[:, pix]
            stat_src[10:15, cols] = f_hi[:, pix]
            bapt[:, s * W2 : s * W2 + K_CLS] = BpT[pix]
            bapt[:, s * W2 + K_CLS : (s + 1) * W2] = ApT[pix]

        # abrep_src: rows 0-20 A', 21-41 B', 42-63 zero, rows 64-127 copy.
        # Band-0 columns of the B' rows are zeroed: the device never writes
        # band 0's A-side PSUM rows (no sub-diagonal tiles there), so this
        # kills the garbage they would otherwise multiply.
        abrep_src = np.zeros((128, p_act), BF16)
        abrep_src[0:K_CLS] = Ap.astype(BF16)
        abrep_src[K_CLS:W2] = Bp.astype(BF16)
        abrep_src[K_CLS:W2, 0:BAND] = 0
        abrep_src[64:] = abrep_src[:64]

        in_maps.append(
            {
                "mov_src": mov_src,
                "stat_src": np.concatenate([stat_src, stat_src], axis=0),
                "bapt": bapt,
                "abrep_src": abrep_src,
            }
        )
    return n_lslot, in_maps


def _get_program(n_lslot):
    key = ("nc", n_lslot)
    if key not in _CACHE:
        _CACHE[key] = _build_program(n_lslot)
    return _CACHE[key]


def _install_profile_hook():
    """Best-effort registration of the axon NTFF profile hook so that
    trace=True works (used by test harness, not the plain kernel path)."""
    import sys
    import types

    if "antenv.axon_hooks" in sys.modules:
        return
    try:
        from trn_agent_boot.trn_boot import _ntff_profile_via_ctypes

        hook = _ntff_profile_via_ctypes("/opt/axon/libaxon_pjrt.so")
        mod = types.ModuleType("antenv.axon_hooks")
        mod.get_axon_ntff_profile_hook = lambda: hook
        sys.modules["antenv.axon_hooks"] = mod
    except Exception:
        pass


def kernel(images, segmentations, ROIs, seg_label, _trace=False, _tmpdir=None):
    from concourse import bass_utils

    n_lslot, in_maps = _host_prep(images, segmentations, ROIs, seg_label)
    nc = _get_program(n_lslot)
    if _trace:
        _install_profile_hook()
        bass_utils.upload_artifacts = lambda tmpdir: f"local:{tmpdir}"
    res = bass_utils.run_bass_kernel_spmd(
        nc, in_maps, list(range(8)), trace=_trace, tmpdir=_tmpdir
    )
    total = 0.0
    for r in res.results:
        total += r["acc_out"].astype(np.float64).sum()
    loss = np.float32(-WEIGHT / N_IMG * total)
    if _trace:
        return np.array([loss], np.float32), res
    return np.array([loss], np.float32)
